# revision 7
# baseline (speedup 1.0000x reference)
"""BlockwiseQuantLinear on 8 trn2 NeuronCores.

y = act_quant_dequant(x) @ (fp8_weight * block_scales).T
  x: [8192, 2048] f32, weight: [2048, 2048] fp8_e4m3fn (OCP), w_scale: [16, 16] f32
  out: [8192, 2048] f32

Strategy (data-parallel over tokens; hardcoded shapes):
  - Host: dequantize the static weight to fp16 (exact wrt reference up to fp16
    rounding) and pre-transpose it K-major so [k_inner=128, k_block, n] SBUF
    tiles DMA with 16KB-contiguous rows. Shard x rows 8 ways.
  - Device (per core, M_sh=1024): per 128-row x tile, in two 1024-wide halves:
    load (512KB), blockwise act quant on DVE (amax over each (1,128) k-block ->
    scale; multiply by 224/amax and cast to TRN fp8e4, which equals the OCP
    e4m3fn quantization at half scale -- TRN's max normal is 240, so the half
    grid keeps values <= 224), dequantize to fp16, DMA-xbar-transpose to
    [k, m]. Then PSUM-accumulated fp16 GEMMs: for each m-tile, 4 psum chains
    (one per 512-wide n chunk) over all 16 k-blocks.
  - Head-latency control: weight chunk 0 is split across all 8 SWDGE queues so
    the first GEMM chain can start ~10us in; a short stream of dummy matmuls
    at t=0 keeps the PE HAM activity monitor busy so real GEMMs start at the
    warm 2.4GHz clock instead of cold 1.2GHz.
  - Queue separation: x loads on the ACT HWDGE queue, xbar transposes alone on
    the SP HWDGE queue (no mode-switch interference), weights + y stores on
    the 8 SWDGE queues.
  - m-tiles 0-1 run c-inner chains (stationary swapped every matmul); m-tiles
    2-7 run kb-outer (stationary reused across the 4 n chunks, 4 interleaved
    psum chains) to amortize LDWEIGHTS.
  - Gather: concatenate the 8 row shards.
"""

import numpy as np
import ml_dtypes

import concourse.bass as bass
import concourse.mybir as mybir
import concourse.tile as tile
from concourse import bacc
from concourse.bass_utils import run_bass_kernel_spmd

P = 128
M, K, N = 8192, 2048, 2048
NCORES = 8
M_SH = M // NCORES            # 1024 rows per core
MT = M_SH // P                # 8 m-tiles per core
KB = K // P                   # 16 k blocks
H = 2                         # halves per m-tile (quant/transpose granularity)
KBH = KB // H                 # 8 k blocks per half
KH_W = KBH * P                # 1024
NCH = 4                       # n chunks of 512
NC_W = N // NCH               # 512
WQ = 4                        # swdge queues; weight chunk 0 split this many ways
EPS = 1e-12
N_WARMUP = 24                 # dummy matmuls to pre-warm the PE clock gate

_cache = {}


def _build():
    nc = bacc.Bacc(None, target_bir_lowering=False, num_swdge_queues=WQ)

    x_in = nc.dram_tensor("x_sh", [M_SH, K], mybir.dt.float32, kind="ExternalInput")
    # [n_chunk, k_inner, k_block, n] -- 16KB contiguous per (c, ki) row
    w_in = nc.dram_tensor(
        "wT", [NCH, P, KB, NC_W], mybir.dt.float16, kind="ExternalInput"
    )
    y_out = nc.dram_tensor("y_sh", [M_SH, N], mybir.dt.float32, kind="ExternalOutput")

    with tile.TileContext(nc) as tc:
        with (
            tc.tile_pool(name="wpool", bufs=1) as wpool,
            tc.tile_pool(name="xpool", bufs=4) as xpool,
            tc.tile_pool(name="qpool", bufs=4) as qpool,
            tc.tile_pool(name="tpool", bufs=MT) as tpool,
            tc.tile_pool(name="spool", bufs=4) as spool,
            tc.tile_pool(name="ypool", bufs=6) as ypool,
            tc.tile_pool(name="ps", bufs=2, space="PSUM") as ps,
        ):
            # PE warmup: junk matmuls with no data deps keep the HAM activity
            # window busy from t~=5us so the first real chain runs at 2.4GHz.
            scratch = spool.tile([P, 5 * P], mybir.dt.float16, name="scratch", bufs=1)
            nc.vector.memset(scratch[:], 0.0)
            # aliases the first rotation of the ps0 chain buffers (PSUM is
            # exactly 4 names x 2 bufs x 2KB = 16KB/partition)
            warm_ps = ps.tile([P, NC_W], mybir.dt.float32, name="ps0")
            for _ in range(N_WARMUP):
                nc.tensor.matmul(
                    warm_ps[:], scratch[:, :P], scratch[:, P:], start=True, stop=True
                )

            # resident weights: 4 tiles of [128, 16, 512] fp16, all on the 8
            # SWDGE queues; chunk 0 split 8 ways so it lands first and the
            # GEMM stream can start as soon as the first xT tiles are up.
            wts = []
            for c in range(NCH):
                wt = wpool.tile([P, KB, NC_W], mybir.dt.float16, name=f"w{c}")
                if c == 0:
                    PSL = P // WQ
                    for q in range(WQ):
                        nc.gpsimd.dma_start(
                            wt[bass.ts(q, PSL), :, :], w_in[c, bass.ts(q, PSL)]
                        )
                else:
                    PSL = P // 2
                    for q in range(2):
                        nc.gpsimd.dma_start(
                            wt[bass.ts(q, PSL), :, :], w_in[c, bass.ts(q, PSL)]
                        )
                wts.append(wt)

            def quant_transpose(mi, h):
                """Load half h of m-tile mi, act-quant it, dequantize to fp16
                and xbar-transpose to [k, m]. Returns the [P, KBH, P] tile."""
                xg = xpool.tile([P, KH_W], mybir.dt.float32, name="xg")
                nc.scalar.dma_start(
                    xg[:], x_in[bass.ts(mi, P), bass.ts(h, KH_W)]
                )
                x3 = xg[:].rearrange("p (kb ki) -> p kb ki", kb=KBH)
                amax = spool.tile([P, KBH], mybir.dt.float32, name=f"amax{h}", bufs=6)
                nc.vector.tensor_reduce(
                    amax[:], x3, axis=mybir.AxisListType.X,
                    op=mybir.AluOpType.max, apply_absolute_value=True,
                )
                amaxp = spool.tile([P, KBH], mybir.dt.float32, name=f"amaxp{h}", bufs=6)
                nc.vector.tensor_scalar_max(amaxp[:], amax[:], EPS)
                rec = spool.tile([P, KBH], mybir.dt.float32, name=f"rec{h}", bufs=6)
                nc.vector.reciprocal(rec[:], amaxp[:])
                inv2 = spool.tile([P, KBH], mybir.dt.float32, name=f"inv2_{h}", bufs=6)
                nc.vector.tensor_scalar_mul(inv2[:], rec[:], 224.0)
                s2 = spool.tile([P, KBH], mybir.dt.float32, name=f"s2_{h}", bufs=6)
                nc.vector.tensor_scalar_mul(s2[:], amaxp[:], 1.0 / 224.0)

                t8 = qpool.tile([P, KH_W], mybir.dt.float8e4, name=f"t8_{h}", bufs=4)
                t83 = t8[:].rearrange("p (kb ki) -> p kb ki", kb=KBH)
                nc.vector.tensor_tensor(
                    t83, x3, inv2[:, :, None].to_broadcast([P, KBH, P]),
                    mybir.AluOpType.mult,
                )
                xdq = qpool.tile([P, KH_W], mybir.dt.float16, name=f"xdq{h}", bufs=4)
                xdq3 = xdq[:].rearrange("p (kb ki) -> p kb ki", kb=KBH)
                nc.vector.tensor_tensor(
                    xdq3, t83, s2[:, :, None].to_broadcast([P, KBH, P]),
                    mybir.AluOpType.mult,
                )
                xT = tpool.tile([P, KBH, P], mybir.dt.float16, name=f"xT{h}")
                nc.sync.dma_start_transpose(xT[:], xdq[:])
                return xT

            def evict(psum, mi, c):
                yc = ypool.tile([P, NC_W], mybir.dt.float32, name="yc")
                nc.any.tensor_copy(yc[:], psum[:])
                nc.gpsimd.dma_start(
                    y_out[bass.ts(mi, P), bass.ts(c, NC_W)], yc[:]
                )

            xTs = {}
            for mi in range(MT):
                xTs[mi] = [quant_transpose(mi, h) for h in range(H)]

                if mi < 2:
                    # c-inner: one psum chain at a time, kb innermost
                    # (stationary changes every matmul, as the weights for
                    # later chunks may still be in flight).
                    for c in range(NCH):
                        psum = ps.tile([P, NC_W], mybir.dt.float32, name=f"ps{c}")
                        for kb in range(KB):
                            h, hk = divmod(kb, KBH)
                            nc.tensor.matmul(
                                psum[:], xTs[mi][h][:, hk, :], wts[c][:, kb, :],
                                start=(kb == 0), stop=(kb == KB - 1),
                            )
                        evict(psum, mi, c)
                else:
                    # kb-outer: 4 interleaved psum chains; the stationary
                    # xT block is reused across the 4 n chunks so LDWEIGHTS
                    # amortizes 4x.
                    psums = [
                        ps.tile([P, NC_W], mybir.dt.float32, name=f"ps{c}")
                        for c in range(NCH)
                    ]
                    for kb in range(KB):
                        h, hk = divmod(kb, KBH)
                        for c in range(NCH):
                            nc.tensor.matmul(
                                psums[c][:], xTs[mi][h][:, hk, :], wts[c][:, kb, :],
                                start=(kb == 0), stop=(kb == KB - 1),
                            )
                    for c in range(NCH):
                        evict(psums[c], mi, c)

    nc.compile()
    return nc


def _prep_weight(weight: np.ndarray, w_scale: np.ndarray) -> np.ndarray:
    w_f32 = weight.astype(np.float32)                     # exact
    ws_full = np.repeat(np.repeat(w_scale.astype(np.float32), P, axis=0), P, axis=1)
    w_deq = (w_f32 * ws_full).astype(np.float16)          # [N, K]
    # w_deq.T[k, n]: k = kb*P + ki, n = c*NC_W + nn -> [c, ki, kb, nn]
    wt = np.ascontiguousarray(
        w_deq.T.reshape(KB, P, NCH, NC_W).transpose(2, 1, 0, 3)
    )
    return wt


def kernel(x: np.ndarray, weight: np.ndarray, w_scale: np.ndarray, _trace: bool = False):
    if "nc" not in _cache:
        _cache["nc"] = _build()
    nc = _cache["nc"]

    weight = np.asarray(weight)
    w_scale = np.asarray(w_scale, dtype=np.float32)
    wt = _prep_weight(weight, w_scale)
    x = np.ascontiguousarray(np.asarray(x), dtype=np.float32)

    in_maps = [
        {"x_sh": x[c * M_SH:(c + 1) * M_SH], "wT": wt}
        for c in range(NCORES)
    ]
    res = run_bass_kernel_spmd(
        nc, in_maps, core_ids=list(range(NCORES)),
        trace=_trace, trace_cores=list(range(NCORES)) if _trace else None,
    )
    y = np.concatenate([res.results[c]["y_sh"] for c in range(NCORES)], axis=0)
    if _trace:
        kernel.last_results = res
    return y
